# revision 18
# baseline (speedup 1.0000x reference)
"""Trainium2 Bass kernel for nn_ClimateAwareDeformableAligner.

Reference semantics (verified numerically):
  - The grid_sample receives input [N, C, H=L, W=1]; the computed grid x-coord
    indexes the size-1 W axis (multiplied by W-1 = 0) and the y-coord is 0 ->
    iy = (L-1)/2 = 2047.5. The whole deformable gather collapses to
        xs[n, c, l] = 0.5 * (xg[n, c, 2047] + xg[n, c, 2048])   (const along l)
    so out0 = x + tanh(gate) * 0.5 * (x[:, 2047, :] + x[:, 2048, :]).
  - offset (second output) needs the full conv stack:
        conv1 -> GroupNorm(4) -> gelu -> conv2 -> gelu -> conv3 -> 10*tanh.

Sharding: data-parallel over batch. 8 cores x 2 batches each; the fused
B*G = 128 rows split as 16 consecutive rows per core.
"""
import ml_dtypes
import numpy as np

import concourse.bacc as bacc
import concourse.bass as bass
import concourse.tile as tile
from concourse import mybir
from concourse.bass_utils import run_bass_kernel_spmd

F32 = mybir.dt.float32
BF16 = mybir.dt.bfloat16
NPBF16 = ml_dtypes.bfloat16
AF = mybir.ActivationFunctionType
ALU = mybir.AluOpType

B, L, C, G, CG, WD = 16, 4096, 512, 8, 64, 4
NCORES = 8
BPC = B // NCORES          # batches per core = 2
NPC = BPC * G              # n rows per core = 16
LT = 512                   # L tile (psum bank)
NLT = L // LT              # 8
EPS = 1e-5


# ---------------------------------------------------------------- host prep
def _host_prep(x, x_ext, w1, b1, gn_g, gn_b, w2, b2, w3, gate):
    xT = np.ascontiguousarray(x.transpose(0, 2, 1))           # [16, 512, 4096]

    xe_pad = np.zeros((B, L + 2, WD), np.float32)
    xe_pad[:, 1:L + 1, :] = x_ext
    xe_col = np.empty((B, 12, L), np.float32)
    for k in range(3):
        for wd in range(WD):
            xe_col[:, k * 4 + wd, :] = xe_pad[:, k:k + L, wd]

    w1x = np.zeros((128, 3, 128), np.float32)                 # [ci, k, co]
    for k in range(3):
        blk = w1[:, :64, k].T
        w1x[:64, k, :64] = blk
        w1x[64:, k, 64:] = blk

    w1e = np.zeros((24, 128), np.float32)
    w1e_flat = np.zeros((12, 64), np.float32)
    for k in range(3):
        for wd in range(WD):
            w1e_flat[k * 4 + wd, :] = w1[:, 64 + wd, k]
    w1e[:12, :64] = w1e_flat
    w1e[12:, 64:] = w1e_flat

    w2b = np.zeros((128, 3, 64), np.float32)
    for k in range(3):
        blk = w2[:, :, k].T
        w2b[:64, k, :32] = blk
        w2b[64:, k, 32:] = blk

    w3b = np.zeros((128, 3, 4), np.float32)
    for k in range(3):
        for m in range(4):
            w3b[32 * m:32 * m + 32, k, m] = w3[0, :, k]

    gmask = np.zeros((128, 8), np.float32)
    expand = np.zeros((8, 128), np.float32)
    for p in range(128):
        gmask[p, p // 16] = 1.0 / 16.0
        expand[p // 16, p] = 1.0

    vecs = np.stack([np.tile(b1, 2), np.tile(gn_g, 2),
                     np.tile(gn_b, 2), np.tile(b2, 4)], axis=1).astype(np.float32)

    halfv = np.full((1, 128), 0.5, np.float32)
    ones2 = np.ones((2, 128), np.float32)
    gate2 = np.asarray(gate, np.float32).reshape(1, 1)

    common = dict(xe=xe_col.astype(NPBF16), w1x=w1x.astype(NPBF16),
                  w1e=w1e.astype(NPBF16), w2=w2b.astype(NPBF16),
                  w3=w3b.astype(NPBF16),
                  gmask=gmask, expand=expand, vecs=vecs,
                  halfv=halfv, ones2=ones2, gate=gate2)
    xTb = xT.astype(NPBF16)
    in_maps = []
    for c in range(NCORES):
        m = dict(common)
        m["x"] = np.ascontiguousarray(x[2 * c:2 * c + 2])
        m["xT"] = np.ascontiguousarray(xTb[2 * c:2 * c + 2])
        in_maps.append(m)
    return in_maps


# ---------------------------------------------------------------- program
def _build_program():
    nc = bacc.Bacc("TRN2", target_bir_lowering=False, debug=False)

    x_d = nc.dram_tensor("x", [BPC, L, C], F32, kind="ExternalInput")[:]
    xT_d = nc.dram_tensor("xT", [BPC, C, L], BF16, kind="ExternalInput")[:]
    xe_d = nc.dram_tensor("xe", [B, 12, L], BF16, kind="ExternalInput")[:]
    w1x_d = nc.dram_tensor("w1x", [128, 3, 128], BF16, kind="ExternalInput")[:]
    w1e_d = nc.dram_tensor("w1e", [24, 128], BF16, kind="ExternalInput")[:]
    w2_d = nc.dram_tensor("w2", [128, 3, 64], BF16, kind="ExternalInput")[:]
    w3_d = nc.dram_tensor("w3", [128, 3, 4], BF16, kind="ExternalInput")[:]
    gm_d = nc.dram_tensor("gmask", [128, 8], F32, kind="ExternalInput")[:]
    ex_d = nc.dram_tensor("expand", [8, 128], F32, kind="ExternalInput")[:]
    vec_d = nc.dram_tensor("vecs", [128, 4], F32, kind="ExternalInput")[:]
    half_d = nc.dram_tensor("halfv", [1, 128], F32, kind="ExternalInput")[:]
    ones2_d = nc.dram_tensor("ones2", [2, 128], F32, kind="ExternalInput")[:]
    gate_d = nc.dram_tensor("gate", [1, 1], F32, kind="ExternalInput")[:]

    out0_d = nc.dram_tensor("out0", [BPC, L, C], F32, kind="ExternalOutput")[:]
    out1_d = nc.dram_tensor("out1", [NPC, L], F32, kind="ExternalOutput")[:]

    with tile.TileContext(nc) as tc:
        with tc.tile_pool(name="wpool", bufs=1) as wpool, \
             tc.tile_pool(name="xtp", bufs=2) as xtp, \
             tc.tile_pool(name="xep", bufs=2) as xep, \
             tc.tile_pool(name="h1p", bufs=4) as h1p, \
             tc.tile_pool(name="h2p", bufs=2) as h2p, \
             tc.tile_pool(name="resp", bufs=4) as resp, \
             tc.tile_pool(name="offp", bufs=2) as offp, \
             tc.tile_pool(name="stp", bufs=2) as stp, \
             tc.tile_pool(name="ps1p", bufs=2, space="PSUM") as ps1p, \
             tc.tile_pool(name="ps2p", bufs=2, space="PSUM") as ps2p, \
             tc.tile_pool(name="ps3p", bufs=2, space="PSUM") as ps3p, \
             tc.tile_pool(name="pstp", bufs=2, space="PSUM") as pstp:

            # ---- constants
            w1x = wpool.tile([128, 3, 128], BF16)
            nc.sync.dma_start(out=w1x, in_=w1x_d)
            w1e = wpool.tile([24, 128], BF16)
            nc.sync.dma_start(out=w1e, in_=w1e_d)
            w2 = wpool.tile([128, 3, 64], BF16)
            nc.sync.dma_start(out=w2, in_=w2_d)
            w3 = wpool.tile([128, 3, 4], BF16)
            nc.sync.dma_start(out=w3, in_=w3_d)
            gmask = wpool.tile([128, 8], F32)
            nc.sync.dma_start(out=gmask, in_=gm_d)
            expand = wpool.tile([8, 128], F32)
            nc.sync.dma_start(out=expand, in_=ex_d)
            vecs = wpool.tile([128, 4], F32)
            nc.sync.dma_start(out=vecs, in_=vec_d)
            halfv = wpool.tile([1, 128], F32)
            nc.sync.dma_start(out=halfv, in_=half_d)
            ones2 = wpool.tile([2, 128], F32)
            nc.sync.dma_start(out=ones2, in_=ones2_d)
            gate = wpool.tile([1, 1], F32)
            nc.sync.dma_start(out=gate, in_=gate_d)
            magic = wpool.tile([8, 1], mybir.dt.uint32)
            nc.vector.memset(magic, 0x5f3759df)

            # ---- residual constant: cvec[b] = 0.5*tanh(gate)*(x[b,2047]+x[b,2048])
            taus = wpool.tile([1, 1], F32)
            nc.scalar.activation(taus, gate, AF.Tanh)
            ps_t = pstp.tile([128, 1], F32, tag="st")
            nc.tensor.matmul(ps_t, halfv, taus, start=True, stop=True)
            tau128 = wpool.tile([128, 1], F32)
            nc.vector.tensor_copy(tau128, ps_t)          # 0.5*tanh(gate) on all parts

            cvecs = []
            for b in range(BPC):
                xrows = wpool.tile([2, C], F32, tag=f"xrows{b}")
                nc.sync.dma_start(out=xrows, in_=x_d[b, 2047:2049, :])
                ps_cv = pstp.tile([128, C], F32, tag="st")
                nc.tensor.matmul(ps_cv, ones2, xrows, start=True, stop=True)
                cv = wpool.tile([128, 4, LT], F32, tag=f"cvec{b}")
                nc.vector.tensor_scalar(cv[:, 0, :], ps_cv, tau128, None, op0=ALU.mult)
                for kk in range(1, 4):
                    nc.gpsimd.tensor_copy(cv[:, kk, :], cv[:, 0, :])
                cvecs.append(cv)

            # residual views: [8, 128, 4, 512] macro-tiles per batch
            xr = [x_d[b].rearrange("(mt ch p) c -> mt p ch c", ch=4, p=128)
                  for b in range(BPC)]
            o0r = [out0_d[b].rearrange("(mt ch p) c -> mt p ch c", ch=4, p=128)
                   for b in range(BPC)]

            def emit_residual(mt_global):
                # rt <- cvec broadcast; then DMA x with inline accumulate (CCE
                # add in the SDMA datapath); store. No compute-engine pass.
                b, mt = mt_global // 8, mt_global % 8
                rt = resp.tile([128, 4, LT], F32, tag="res")
                nc.gpsimd.tensor_copy(rt, cvecs[b])
                nc.gpsimd.dma_start(out=rt, in_=xr[b][mt], accum_op=ALU.add)
                nc.scalar.dma_start(out=o0r[b][mt], in_=rt)

            def conv1_pair(p):
                """conv1 + bn_stats for pair p -> (h1 tile, stats tile)."""
                b_idx = p // 4
                ch0 = 128 * (p % 4)
                j0 = 2 * p
                xt = xtp.tile([128, L + 2], BF16, tag="xt")
                nc.gpsimd.memset(xt[:, 0:1], 0.0)
                nc.gpsimd.memset(xt[:, L + 1:L + 2], 0.0)
                nc.sync.dma_start(out=xt[:, 1:L + 1], in_=xT_d[b_idx, ch0:ch0 + 128, :])
                xe = xep.tile([24, L], BF16, tag="xe")
                nc.sync.dma_start(out=xe, in_=xe_d[j0:j0 + 2].rearrange("a b l -> (a b) l"))
                h1t = h1p.tile([128, L + 2], BF16, tag="h1")
                nc.gpsimd.memset(h1t[:, 0:1], 0.0)
                nc.gpsimd.memset(h1t[:, L + 1:L + 2], 0.0)
                stats = stp.tile([128, NLT, 6], F32, tag="stats")
                for lt in range(NLT):
                    ps = ps1p.tile([128, LT], F32, tag="ps1")
                    for k in range(3):
                        nc.tensor.matmul(ps, w1x[:, k, :],
                                         xt[:, lt * LT + k: lt * LT + k + LT],
                                         start=(k == 0), stop=False)
                    nc.tensor.matmul(ps, w1e, xe[:, lt * LT:(lt + 1) * LT],
                                     start=False, stop=True)
                    nc.vector.bn_stats(stats[:, lt, :], ps)
                    nc.vector.tensor_copy(h1t[:, 1 + lt * LT: 1 + (lt + 1) * LT], ps)
                return h1t, stats

            def stats_gelu(h1t, stats):
                """GroupNorm affine (b1 folded) + gelu, in place on h1t."""
                mv = stp.tile([128, 2], F32, tag="mv")
                nc.vector.bn_aggr(mv, stats)
                svec = stp.tile([128, 2], F32, tag="svec")
                nc.vector.tensor_add(svec[:, 0:1], mv[:, 0:1], vecs[:, 0:1])
                sq = stp.tile([128, 1], F32, tag="sq")
                nc.vector.tensor_mul(sq, svec[:, 0:1], svec[:, 0:1])
                nc.vector.tensor_add(svec[:, 1:2], mv[:, 1:2], sq)
                psg = pstp.tile([8, 2], F32, tag="st")
                nc.tensor.matmul(psg, gmask, svec, start=True, stop=True)
                gv = stp.tile([8, 2], F32, tag="gv")
                nc.vector.tensor_copy(gv, psg)
                musq = stp.tile([8, 1], F32, tag="musq")
                nc.vector.tensor_mul(musq, gv[:, 0:1], gv[:, 0:1])
                varg = stp.tile([8, 1], F32, tag="varg")
                nc.vector.tensor_sub(varg, gv[:, 1:2], musq)
                # rstd = 1/sqrt(varg + eps), table-free (Quake seed + Newton)
                nc.vector.tensor_scalar(varg, varg, EPS, None, op0=ALU.add)
                vh = stp.tile([8, 1], F32, tag="vh")
                nc.vector.tensor_scalar(vh, varg, 0.5, None, op0=ALU.mult)
                yb = stp.tile([8, 1], mybir.dt.uint32, tag="yb")
                nc.vector.tensor_scalar(yb, varg.bitcast(mybir.dt.uint32), 1, None,
                                        op0=ALU.logical_shift_right)
                nc.vector.tensor_sub(yb, magic, yb)
                rhs2 = stp.tile([8, 2], F32, tag="rhs2")
                y = yb.bitcast(F32)
                tq = stp.tile([8, 1], F32, tag="tq")
                for it in range(3):
                    dst = rhs2[:, 0:1] if it == 2 else y
                    nc.vector.tensor_mul(tq, y, y)
                    nc.vector.tensor_mul(tq, tq, vh)
                    nc.vector.tensor_scalar(tq, tq, 1.5, -1.0,
                                            op0=ALU.subtract, op1=ALU.mult)
                    nc.vector.tensor_mul(dst, y, tq)
                nc.vector.tensor_copy(rhs2[:, 1:2], gv[:, 0:1])
                psp = pstp.tile([128, 2], F32, tag="st")
                nc.tensor.matmul(psp, expand, rhs2, start=True, stop=True)
                ap_ = stp.tile([128, 1], F32, tag="ap_")
                bp_ = stp.tile([128, 1], F32, tag="bp_")
                tmp = stp.tile([128, 1], F32, tag="tmp")
                nc.vector.tensor_mul(ap_, psp[:, 0:1], vecs[:, 1:2])
                nc.vector.tensor_sub(tmp, vecs[:, 0:1], psp[:, 1:2])
                nc.vector.tensor_mul(tmp, tmp, ap_)
                nc.vector.tensor_add(bp_, tmp, vecs[:, 2:3])
                nc.scalar.activation(h1t[:, 1:L + 1], h1t[:, 1:L + 1], AF.Gelu,
                                     bias=bp_, scale=ap_)

            def conv3_lt(h2q, q, lt):
                ps3 = ps3p.tile([4, LT], F32, tag="ps3")
                for k in range(3):
                    nc.tensor.matmul(ps3, w3[:, k, :],
                                     h2q[:, lt * LT + k: lt * LT + k + LT],
                                     start=(k == 0), stop=(k == 2))
                ot = offp.tile([4, LT], F32, tag="off")
                nc.scalar.activation(ot, ps3, AF.Tanh)
                nc.gpsimd.tensor_scalar(ot, ot, 10.0, None, op0=ALU.mult)
                nc.sync.dma_start(out=out1_d[4 * q:4 * q + 4, lt * LT:(lt + 1) * LT],
                                  in_=ot)

            def phase1(q):
                """residual tiles + conv1 + GN/gelu for quad q's two pairs."""
                for mt in range(4 * q, 4 * q + 4):
                    emit_residual(mt)
                h1a, sta = conv1_pair(2 * q)
                h1b, stb = conv1_pair(2 * q + 1)
                stats_gelu(h1a, sta)
                stats_gelu(h1b, stb)
                return h1a, h1b

            def phase2(q, h1a, h1b):
                h2q = h2p.tile([128, L + 2], BF16, tag="h2")
                nc.gpsimd.memset(h2q[:, 0:1], 0.0)
                nc.gpsimd.memset(h2q[:, L + 1:L + 2], 0.0)
                for lt in range(NLT):
                    ps2 = ps2p.tile([128, LT], F32, tag="ps2")
                    for half, h1t in enumerate((h1a, h1b)):
                        tp = (0, 64 * half) if half else None
                        for k in range(3):
                            nc.tensor.matmul(ps2[64 * half:64 * half + 64, :],
                                             w2[:, k, :],
                                             h1t[:, lt * LT + k: lt * LT + k + LT],
                                             start=(k == 0), stop=(k == 2),
                                             tile_position=tp)
                    nc.scalar.activation(h2q[:, 1 + lt * LT: 1 + (lt + 1) * LT],
                                         ps2, AF.Gelu, bias=vecs[:, 3:4], scale=1.0)
                    if lt >= 1:
                        conv3_lt(h2q, q, lt - 1)
                conv3_lt(h2q, q, NLT - 1)

            # software pipeline: conv1 of quad q+1 is emitted before the
            # conv2/conv3 stage of quad q so the PE stream stays dense.
            h1s = {0: phase1(0)}
            for q in range(4):
                if q + 1 < 4:
                    h1s[q + 1] = phase1(q + 1)
                phase2(q, *h1s.pop(q))

    nc.finalize()
    return nc


_CACHE = {}


def _get_program():
    if "nc" not in _CACHE:
        _CACHE["nc"] = _build_program()
    return _CACHE["nc"]


def kernel(x, x_ext, w1, b1, gn_g, gn_b, w2, b2, w3, gate, trace=False):
    x = np.asarray(x, np.float32)
    in_maps = _host_prep(x, np.asarray(x_ext, np.float32),
                         np.asarray(w1, np.float32), np.asarray(b1, np.float32),
                         np.asarray(gn_g, np.float32), np.asarray(gn_b, np.float32),
                         np.asarray(w2, np.float32), np.asarray(b2, np.float32),
                         np.asarray(w3, np.float32), np.asarray(gate, np.float32))
    nc = _get_program()
    res = run_bass_kernel_spmd(nc, in_maps, core_ids=list(range(NCORES)),
                               trace=trace)
    out0 = np.concatenate([res.results[c]["out0"] for c in range(NCORES)], axis=0)
    out1 = np.concatenate([res.results[c]["out1"] for c in range(NCORES)], axis=0)
    out1 = out1.reshape(B * G, 1, L)
    if trace:
        kernel.last_result = res
    return out0, out1


# revision 22
# speedup vs baseline: 1.6139x; 1.6139x over previous
"""Trainium2 Bass kernel for nn_ClimateAwareDeformableAligner.

Reference semantics (verified numerically):
  - The grid_sample receives input [N, C, H=L, W=1]; the computed grid x-coord
    indexes the size-1 W axis (multiplied by W-1 = 0) and the y-coord is 0 ->
    iy = (L-1)/2 = 2047.5. The whole deformable gather collapses to
        xs[n, c, l] = 0.5 * (xg[n, c, 2047] + xg[n, c, 2048])   (const along l)
    so out0 = x + tanh(gate) * 0.5 * (x[:, 2047, :] + x[:, 2048, :]).
  - offset (second output) needs the full conv stack:
        conv1 -> GroupNorm(4) -> gelu -> conv2 -> gelu -> conv3 -> 10*tanh.

Sharding: data-parallel over batch. 8 cores x 2 batches each; the fused
B*G = 128 rows split as 16 consecutive rows per core.
"""
import ml_dtypes
import numpy as np

import concourse.bacc as bacc
import concourse.bass as bass
import concourse.tile as tile
from concourse import mybir
from concourse.bass_utils import run_bass_kernel_spmd

F32 = mybir.dt.float32
BF16 = mybir.dt.bfloat16
NPBF16 = ml_dtypes.bfloat16
AF = mybir.ActivationFunctionType
ALU = mybir.AluOpType

B, L, C, G, CG, WD = 16, 4096, 512, 8, 64, 4
NCORES = 8
BPC = B // NCORES          # batches per core = 2
NPC = BPC * G              # n rows per core = 16
LT = 512                   # L tile (psum bank)
NLT = L // LT              # 8
EPS = 1e-5


# ---------------------------------------------------------------- host prep
def _host_prep(x, x_ext, w1, b1, gn_g, gn_b, w2, b2, w3, gate):
    xT = np.ascontiguousarray(x.transpose(0, 2, 1))           # [16, 512, 4096]

    xe_pad = np.zeros((B, L + 2, WD), np.float32)
    xe_pad[:, 1:L + 1, :] = x_ext
    xe_col = np.empty((B, 12, L), np.float32)
    for k in range(3):
        for wd in range(WD):
            xe_col[:, k * 4 + wd, :] = xe_pad[:, k:k + L, wd]

    w1x = np.zeros((128, 3, 128), np.float32)                 # [ci, k, co]
    for k in range(3):
        blk = w1[:, :64, k].T
        w1x[:64, k, :64] = blk
        w1x[64:, k, 64:] = blk

    w1e = np.zeros((24, 128), np.float32)
    w1e_flat = np.zeros((12, 64), np.float32)
    for k in range(3):
        for wd in range(WD):
            w1e_flat[k * 4 + wd, :] = w1[:, 64 + wd, k]
    w1e[:12, :64] = w1e_flat
    w1e[12:, 64:] = w1e_flat

    w2b = np.zeros((128, 3, 64), np.float32)
    for k in range(3):
        blk = w2[:, :, k].T
        w2b[:64, k, :32] = blk
        w2b[64:, k, 32:] = blk

    w3b = np.zeros((128, 3, 4), np.float32)
    for k in range(3):
        for m in range(4):
            w3b[32 * m:32 * m + 32, k, m] = w3[0, :, k]

    gmask = np.zeros((128, 8), np.float32)
    expand = np.zeros((8, 128), np.float32)
    for p in range(128):
        gmask[p, p // 16] = 1.0 / 16.0
        expand[p // 16, p] = 1.0

    vecs = np.stack([np.tile(b1, 2), np.tile(gn_g, 2),
                     np.tile(gn_b, 2), np.tile(b2, 4)], axis=1).astype(np.float32)

    halfv = np.full((1, 128), 0.5, np.float32)
    ones2 = np.ones((2, 128), np.float32)
    gate2 = np.asarray(gate, np.float32).reshape(1, 1)

    common = dict(xe=xe_col.astype(NPBF16), w1x=w1x.astype(NPBF16),
                  w1e=w1e.astype(NPBF16), w2=w2b.astype(NPBF16),
                  w3=w3b.astype(NPBF16),
                  gmask=gmask, expand=expand, vecs=vecs,
                  halfv=halfv, ones2=ones2, gate=gate2)
    xTb = xT.astype(NPBF16)
    in_maps = []
    for c in range(NCORES):
        m = dict(common)
        m["x"] = np.ascontiguousarray(x[2 * c:2 * c + 2])
        m["xT"] = np.ascontiguousarray(xTb[2 * c:2 * c + 2])
        in_maps.append(m)
    return in_maps


# ---------------------------------------------------------------- program
def _build_program():
    nc = bacc.Bacc("TRN2", target_bir_lowering=False, debug=False)

    x_d = nc.dram_tensor("x", [BPC, L, C], F32, kind="ExternalInput")[:]
    xT_d = nc.dram_tensor("xT", [BPC, C, L], BF16, kind="ExternalInput")[:]
    xe_d = nc.dram_tensor("xe", [B, 12, L], BF16, kind="ExternalInput")[:]
    w1x_d = nc.dram_tensor("w1x", [128, 3, 128], BF16, kind="ExternalInput")[:]
    w1e_d = nc.dram_tensor("w1e", [24, 128], BF16, kind="ExternalInput")[:]
    w2_d = nc.dram_tensor("w2", [128, 3, 64], BF16, kind="ExternalInput")[:]
    w3_d = nc.dram_tensor("w3", [128, 3, 4], BF16, kind="ExternalInput")[:]
    gm_d = nc.dram_tensor("gmask", [128, 8], F32, kind="ExternalInput")[:]
    ex_d = nc.dram_tensor("expand", [8, 128], F32, kind="ExternalInput")[:]
    vec_d = nc.dram_tensor("vecs", [128, 4], F32, kind="ExternalInput")[:]
    half_d = nc.dram_tensor("halfv", [1, 128], F32, kind="ExternalInput")[:]
    ones2_d = nc.dram_tensor("ones2", [2, 128], F32, kind="ExternalInput")[:]
    gate_d = nc.dram_tensor("gate", [1, 1], F32, kind="ExternalInput")[:]

    out0_d = nc.dram_tensor("out0", [BPC, L, C], F32, kind="ExternalOutput")[:]
    out1_d = nc.dram_tensor("out1", [NPC, L], F32, kind="ExternalOutput")[:]

    with tile.TileContext(nc) as tc:
        with tc.tile_pool(name="wpool", bufs=1) as wpool, \
             tc.tile_pool(name="xtp", bufs=2) as xtp, \
             tc.tile_pool(name="xep", bufs=2) as xep, \
             tc.tile_pool(name="h1p", bufs=4) as h1p, \
             tc.tile_pool(name="h2p", bufs=2) as h2p, \
             tc.tile_pool(name="resp", bufs=4) as resp, \
             tc.tile_pool(name="offp", bufs=2) as offp, \
             tc.tile_pool(name="stp", bufs=2) as stp, \
             tc.tile_pool(name="ps1p", bufs=2, space="PSUM") as ps1p, \
             tc.tile_pool(name="ps2p", bufs=2, space="PSUM") as ps2p, \
             tc.tile_pool(name="ps3p", bufs=2, space="PSUM") as ps3p, \
             tc.tile_pool(name="pstp", bufs=2, space="PSUM") as pstp:

            # ---- constants
            w1x = wpool.tile([128, 3, 128], BF16)
            nc.sync.dma_start(out=w1x, in_=w1x_d)
            w1e = wpool.tile([24, 128], BF16)
            nc.sync.dma_start(out=w1e, in_=w1e_d)
            w2 = wpool.tile([128, 3, 64], BF16)
            nc.sync.dma_start(out=w2, in_=w2_d)
            w3 = wpool.tile([128, 3, 4], BF16)
            nc.sync.dma_start(out=w3, in_=w3_d)
            gmask = wpool.tile([128, 8], F32)
            nc.sync.dma_start(out=gmask, in_=gm_d)
            expand = wpool.tile([8, 128], F32)
            nc.sync.dma_start(out=expand, in_=ex_d)
            vecs = wpool.tile([128, 4], F32)
            nc.sync.dma_start(out=vecs, in_=vec_d)
            halfv = wpool.tile([1, 128], F32)
            nc.sync.dma_start(out=halfv, in_=half_d)
            ones2 = wpool.tile([2, 128], F32)
            nc.sync.dma_start(out=ones2, in_=ones2_d)
            gate = wpool.tile([1, 1], F32)
            nc.sync.dma_start(out=gate, in_=gate_d)
            magic = wpool.tile([8, 1], mybir.dt.uint32)
            nc.vector.memset(magic, 0x5f3759df)

            # ---- residual constant: cvec[b] = 0.5*tanh(gate)*(x[b,2047]+x[b,2048])
            taus = wpool.tile([1, 1], F32)
            nc.scalar.activation(taus, gate, AF.Tanh)
            ps_t = pstp.tile([128, 1], F32, tag="st")
            nc.tensor.matmul(ps_t, halfv, taus, start=True, stop=True)
            tau128 = wpool.tile([128, 1], F32)
            nc.vector.tensor_copy(tau128, ps_t)          # 0.5*tanh(gate) on all parts

            cvecs = []
            for b in range(BPC):
                xrows = wpool.tile([2, C], F32, tag=f"xrows{b}")
                nc.sync.dma_start(out=xrows, in_=x_d[b, 2047:2049, :])
                ps_cv = pstp.tile([128, C], F32, tag="st")
                nc.tensor.matmul(ps_cv, ones2, xrows, start=True, stop=True)
                cv = wpool.tile([128, 4, LT], F32, tag=f"cvec{b}")
                nc.vector.tensor_scalar(cv[:, 0, :], ps_cv, tau128, None, op0=ALU.mult)
                for kk in range(1, 4):
                    nc.vector.tensor_copy(cv[:, kk, :], cv[:, 0, :])
                cvecs.append(cv)

            # residual views: [8, 128, 4, 512] macro-tiles per batch
            xr = [x_d[b].rearrange("(mt ch p) c -> mt p ch c", ch=4, p=128)
                  for b in range(BPC)]
            o0r = [out0_d[b].rearrange("(mt ch p) c -> mt p ch c", ch=4, p=128)
                   for b in range(BPC)]

            def emit_residual(mt_global):
                # rt <- cvec broadcast; then DMA x with inline accumulate (CCE
                # add in the SDMA datapath); store. No compute-engine pass.
                b, mt = mt_global // 8, mt_global % 8
                rt = resp.tile([128, 4, LT], F32, tag="res")
                nc.vector.tensor_copy(rt, cvecs[b])
                nc.gpsimd.dma_start(out=rt, in_=xr[b][mt], accum_op=ALU.add)
                nc.scalar.dma_start(out=o0r[b][mt], in_=rt)

            def conv1_pair(p):
                """conv1 + bn_stats for pair p -> (h1 tile, stats tile)."""
                b_idx = p // 4
                ch0 = 128 * (p % 4)
                j0 = 2 * p
                xt = xtp.tile([128, L + 2], BF16, tag="xt")
                nc.gpsimd.memset(xt[:, 0:1], 0.0)
                nc.gpsimd.memset(xt[:, L + 1:L + 2], 0.0)
                nc.sync.dma_start(out=xt[:, 1:L + 1], in_=xT_d[b_idx, ch0:ch0 + 128, :])
                xe = xep.tile([24, L], BF16, tag="xe")
                nc.sync.dma_start(out=xe, in_=xe_d[j0:j0 + 2].rearrange("a b l -> (a b) l"))
                h1t = h1p.tile([128, L + 2], BF16, tag="h1")
                nc.gpsimd.memset(h1t[:, 0:1], 0.0)
                nc.gpsimd.memset(h1t[:, L + 1:L + 2], 0.0)
                stats = stp.tile([128, NLT, 6], F32, tag="stats")
                for lt in range(NLT):
                    ps = ps1p.tile([128, LT], F32, tag="ps1")
                    for k in range(3):
                        nc.tensor.matmul(ps, w1x[:, k, :],
                                         xt[:, lt * LT + k: lt * LT + k + LT],
                                         start=(k == 0), stop=False)
                    nc.tensor.matmul(ps, w1e, xe[:, lt * LT:(lt + 1) * LT],
                                     start=False, stop=True)
                    dst = h1t[:, 1 + lt * LT: 1 + (lt + 1) * LT]
                    if lt % 2 == 0:
                        nc.vector.tensor_copy(dst, ps)
                    else:
                        nc.scalar.activation(dst, ps, AF.Copy)
                    nc.vector.bn_stats(stats[:, lt, :], dst)
                return h1t, stats

            def stats_gelu(h1t, stats):
                """GroupNorm affine (b1 folded) + gelu, in place on h1t."""
                mv = stp.tile([128, 2], F32, tag="mv")
                nc.vector.bn_aggr(mv, stats)
                svec = stp.tile([128, 2], F32, tag="svec")
                nc.vector.tensor_add(svec[:, 0:1], mv[:, 0:1], vecs[:, 0:1])
                sq = stp.tile([128, 1], F32, tag="sq")
                nc.vector.tensor_mul(sq, svec[:, 0:1], svec[:, 0:1])
                nc.vector.tensor_add(svec[:, 1:2], mv[:, 1:2], sq)
                psg = pstp.tile([8, 2], F32, tag="st")
                nc.tensor.matmul(psg, gmask, svec, start=True, stop=True)
                gv = stp.tile([8, 2], F32, tag="gv")
                nc.vector.tensor_copy(gv, psg)
                musq = stp.tile([8, 1], F32, tag="musq")
                nc.vector.tensor_mul(musq, gv[:, 0:1], gv[:, 0:1])
                varg = stp.tile([8, 1], F32, tag="varg")
                nc.vector.tensor_sub(varg, gv[:, 1:2], musq)
                # rstd = 1/sqrt(varg + eps), table-free (Quake seed + Newton)
                nc.vector.tensor_scalar(varg, varg, EPS, None, op0=ALU.add)
                vh = stp.tile([8, 1], F32, tag="vh")
                nc.vector.tensor_scalar(vh, varg, 0.5, None, op0=ALU.mult)
                yb = stp.tile([8, 1], mybir.dt.uint32, tag="yb")
                nc.vector.tensor_scalar(yb, varg.bitcast(mybir.dt.uint32), 1, None,
                                        op0=ALU.logical_shift_right)
                nc.vector.tensor_sub(yb, magic, yb)
                rhs2 = stp.tile([8, 2], F32, tag="rhs2")
                y = yb.bitcast(F32)
                tq = stp.tile([8, 1], F32, tag="tq")
                for it in range(3):
                    dst = rhs2[:, 0:1] if it == 2 else y
                    nc.vector.tensor_mul(tq, y, y)
                    nc.vector.tensor_mul(tq, tq, vh)
                    nc.vector.tensor_scalar(tq, tq, 1.5, -1.0,
                                            op0=ALU.subtract, op1=ALU.mult)
                    nc.vector.tensor_mul(dst, y, tq)
                nc.vector.tensor_copy(rhs2[:, 1:2], gv[:, 0:1])
                psp = pstp.tile([128, 2], F32, tag="st")
                nc.tensor.matmul(psp, expand, rhs2, start=True, stop=True)
                ap_ = stp.tile([128, 1], F32, tag="ap_")
                bp_ = stp.tile([128, 1], F32, tag="bp_")
                tmp = stp.tile([128, 1], F32, tag="tmp")
                nc.vector.tensor_mul(ap_, psp[:, 0:1], vecs[:, 1:2])
                nc.vector.tensor_sub(tmp, vecs[:, 0:1], psp[:, 1:2])
                nc.vector.tensor_mul(tmp, tmp, ap_)
                nc.vector.tensor_add(bp_, tmp, vecs[:, 2:3])
                nc.scalar.activation(h1t[:, 1:L + 1], h1t[:, 1:L + 1], AF.Gelu,
                                     bias=bp_, scale=ap_)

            def conv3_lt(h2q, q, lt):
                ps3 = ps3p.tile([4, LT], F32, tag="ps3")
                for k in range(3):
                    nc.tensor.matmul(ps3, w3[:, k, :],
                                     h2q[:, lt * LT + k: lt * LT + k + LT],
                                     start=(k == 0), stop=(k == 2))
                ot = offp.tile([4, LT], F32, tag="off")
                nc.scalar.activation(ot, ps3, AF.Tanh)
                nc.vector.tensor_scalar(ot, ot, 10.0, None, op0=ALU.mult)
                nc.sync.dma_start(out=out1_d[4 * q:4 * q + 4, lt * LT:(lt + 1) * LT],
                                  in_=ot)

            def phase1(q):
                """residual tiles + conv1 + GN/gelu for quad q's two pairs."""
                for mt in range(4 * q, 4 * q + 4):
                    emit_residual(mt)
                h1a, sta = conv1_pair(2 * q)
                h1b, stb = conv1_pair(2 * q + 1)
                stats_gelu(h1a, sta)
                stats_gelu(h1b, stb)
                return h1a, h1b

            def phase2(q, h1a, h1b):
                h2q = h2p.tile([128, L + 2], BF16, tag="h2")
                nc.gpsimd.memset(h2q[:, 0:1], 0.0)
                nc.gpsimd.memset(h2q[:, L + 1:L + 2], 0.0)
                for lt in range(NLT):
                    ps2 = ps2p.tile([128, LT], F32, tag="ps2")
                    for half, h1t in enumerate((h1a, h1b)):
                        tp = (0, 64 * half) if half else None
                        for k in range(3):
                            nc.tensor.matmul(ps2[64 * half:64 * half + 64, :],
                                             w2[:, k, :],
                                             h1t[:, lt * LT + k: lt * LT + k + LT],
                                             start=(k == 0), stop=(k == 2),
                                             tile_position=tp)
                    nc.scalar.activation(h2q[:, 1 + lt * LT: 1 + (lt + 1) * LT],
                                         ps2, AF.Gelu, bias=vecs[:, 3:4], scale=1.0)
                    if lt >= 1:
                        conv3_lt(h2q, q, lt - 1)
                conv3_lt(h2q, q, NLT - 1)

            # software pipeline: conv1 of quad q+1 is emitted before the
            # conv2/conv3 stage of quad q so the PE stream stays dense.
            h1s = {0: phase1(0)}
            for q in range(4):
                if q + 1 < 4:
                    h1s[q + 1] = phase1(q + 1)
                phase2(q, *h1s.pop(q))

    nc.finalize()
    return nc


_CACHE = {}


def _get_program():
    if "nc" not in _CACHE:
        _CACHE["nc"] = _build_program()
    return _CACHE["nc"]


def kernel(x, x_ext, w1, b1, gn_g, gn_b, w2, b2, w3, gate, trace=False):
    x = np.asarray(x, np.float32)
    in_maps = _host_prep(x, np.asarray(x_ext, np.float32),
                         np.asarray(w1, np.float32), np.asarray(b1, np.float32),
                         np.asarray(gn_g, np.float32), np.asarray(gn_b, np.float32),
                         np.asarray(w2, np.float32), np.asarray(b2, np.float32),
                         np.asarray(w3, np.float32), np.asarray(gate, np.float32))
    nc = _get_program()
    res = run_bass_kernel_spmd(nc, in_maps, core_ids=list(range(NCORES)),
                               trace=trace)
    out0 = np.concatenate([res.results[c]["out0"] for c in range(NCORES)], axis=0)
    out1 = np.concatenate([res.results[c]["out1"] for c in range(NCORES)], axis=0)
    out1 = out1.reshape(B * G, 1, L)
    if trace:
        kernel.last_result = res
    return out0, out1


# revision 25
# speedup vs baseline: 1.6756x; 1.0383x over previous
"""Trainium2 Bass kernel for nn_ClimateAwareDeformableAligner.

Reference semantics (verified numerically):
  - The grid_sample receives input [N, C, H=L, W=1]; the computed grid x-coord
    indexes the size-1 W axis (multiplied by W-1 = 0) and the y-coord is 0 ->
    iy = (L-1)/2 = 2047.5. The whole deformable gather collapses to
        xs[n, c, l] = 0.5 * (xg[n, c, 2047] + xg[n, c, 2048])   (const along l)
    so out0 = x + tanh(gate) * 0.5 * (x[:, 2047, :] + x[:, 2048, :]).
  - offset (second output) needs the full conv stack:
        conv1 -> GroupNorm(4) -> gelu -> conv2 -> gelu -> conv3 -> 10*tanh.

Sharding: data-parallel over batch. 8 cores x 2 batches each; the fused
B*G = 128 rows split as 16 consecutive rows per core.
"""
import ml_dtypes
import numpy as np

import concourse.bacc as bacc
import concourse.bass as bass
import concourse.tile as tile
from concourse import mybir
from concourse.bass_utils import run_bass_kernel_spmd

F32 = mybir.dt.float32
BF16 = mybir.dt.bfloat16
NPBF16 = ml_dtypes.bfloat16
AF = mybir.ActivationFunctionType
ALU = mybir.AluOpType

B, L, C, G, CG, WD = 16, 4096, 512, 8, 64, 4
NCORES = 8
BPC = B // NCORES          # batches per core = 2
NPC = BPC * G              # n rows per core = 16
LT = 512                   # L tile (psum bank)
NLT = L // LT              # 8
EPS = 1e-5


# ---------------------------------------------------------------- host prep
def _host_prep(x, x_ext, w1, b1, gn_g, gn_b, w2, b2, w3, gate):
    xT = np.ascontiguousarray(x.transpose(0, 2, 1))           # [16, 512, 4096]

    xe_pad = np.zeros((B, L + 2, WD), np.float32)
    xe_pad[:, 1:L + 1, :] = x_ext
    xe_col = np.empty((B, 12, L), np.float32)
    for k in range(3):
        for wd in range(WD):
            xe_col[:, k * 4 + wd, :] = xe_pad[:, k:k + L, wd]

    w1x = np.zeros((128, 3, 128), np.float32)                 # [ci, k, co]
    for k in range(3):
        blk = w1[:, :64, k].T
        w1x[:64, k, :64] = blk
        w1x[64:, k, 64:] = blk

    w1e = np.zeros((24, 128), np.float32)
    w1e_flat = np.zeros((12, 64), np.float32)
    for k in range(3):
        for wd in range(WD):
            w1e_flat[k * 4 + wd, :] = w1[:, 64 + wd, k]
    w1e[:12, :64] = w1e_flat
    w1e[12:, 64:] = w1e_flat

    w2b = np.zeros((128, 3, 64), np.float32)
    for k in range(3):
        blk = w2[:, :, k].T
        w2b[:64, k, :32] = blk
        w2b[64:, k, 32:] = blk

    w3b = np.zeros((128, 3, 4), np.float32)
    for k in range(3):
        for m in range(4):
            w3b[32 * m:32 * m + 32, k, m] = w3[0, :, k]

    gmask = np.zeros((128, 8), np.float32)
    expand = np.zeros((8, 128), np.float32)
    for p in range(128):
        gmask[p, p // 16] = 1.0 / 16.0
        expand[p // 16, p] = 1.0

    vecs = np.stack([np.tile(b1, 2), np.tile(gn_g, 2),
                     np.tile(gn_b, 2), np.tile(b2, 4)], axis=1).astype(np.float32)

    halfv = np.full((1, 128), 0.5, np.float32)
    ones2 = np.ones((2, 128), np.float32)
    gate2 = np.asarray(gate, np.float32).reshape(1, 1)

    common = dict(xe=xe_col.astype(NPBF16), w1x=w1x.astype(NPBF16),
                  w1e=w1e.astype(NPBF16), w2=w2b.astype(NPBF16),
                  w3=w3b.astype(NPBF16),
                  gmask=gmask, expand=expand, vecs=vecs,
                  halfv=halfv, ones2=ones2, gate=gate2)
    xTb = xT.astype(NPBF16)
    in_maps = []
    for c in range(NCORES):
        m = dict(common)
        m["x"] = np.ascontiguousarray(x[2 * c:2 * c + 2])
        m["xT"] = np.ascontiguousarray(xTb[2 * c:2 * c + 2])
        in_maps.append(m)
    return in_maps


# ---------------------------------------------------------------- program
def _build_program():
    nc = bacc.Bacc("TRN2", target_bir_lowering=False, debug=False)

    x_d = nc.dram_tensor("x", [BPC, L, C], F32, kind="ExternalInput")[:]
    xT_d = nc.dram_tensor("xT", [BPC, C, L], BF16, kind="ExternalInput")[:]
    xe_d = nc.dram_tensor("xe", [B, 12, L], BF16, kind="ExternalInput")[:]
    w1x_d = nc.dram_tensor("w1x", [128, 3, 128], BF16, kind="ExternalInput")[:]
    w1e_d = nc.dram_tensor("w1e", [24, 128], BF16, kind="ExternalInput")[:]
    w2_d = nc.dram_tensor("w2", [128, 3, 64], BF16, kind="ExternalInput")[:]
    w3_d = nc.dram_tensor("w3", [128, 3, 4], BF16, kind="ExternalInput")[:]
    gm_d = nc.dram_tensor("gmask", [128, 8], F32, kind="ExternalInput")[:]
    ex_d = nc.dram_tensor("expand", [8, 128], F32, kind="ExternalInput")[:]
    vec_d = nc.dram_tensor("vecs", [128, 4], F32, kind="ExternalInput")[:]
    half_d = nc.dram_tensor("halfv", [1, 128], F32, kind="ExternalInput")[:]
    ones2_d = nc.dram_tensor("ones2", [2, 128], F32, kind="ExternalInput")[:]
    gate_d = nc.dram_tensor("gate", [1, 1], F32, kind="ExternalInput")[:]

    out0_d = nc.dram_tensor("out0", [BPC, L, C], F32, kind="ExternalOutput")[:]
    out1_d = nc.dram_tensor("out1", [NPC, L], F32, kind="ExternalOutput")[:]

    with tile.TileContext(nc) as tc:
        with tc.tile_pool(name="wpool", bufs=1) as wpool, \
             tc.tile_pool(name="xtp", bufs=2) as xtp, \
             tc.tile_pool(name="xep", bufs=2) as xep, \
             tc.tile_pool(name="h1p", bufs=4) as h1p, \
             tc.tile_pool(name="h2p", bufs=2) as h2p, \
             tc.tile_pool(name="resp", bufs=4) as resp, \
             tc.tile_pool(name="offp", bufs=2) as offp, \
             tc.tile_pool(name="stp", bufs=2) as stp, \
             tc.tile_pool(name="ps1p", bufs=2, space="PSUM") as ps1p, \
             tc.tile_pool(name="ps2p", bufs=2, space="PSUM") as ps2p, \
             tc.tile_pool(name="ps3p", bufs=2, space="PSUM") as ps3p, \
             tc.tile_pool(name="pstp", bufs=2, space="PSUM") as pstp:

            # ---- constants
            w1x = wpool.tile([128, 3, 128], BF16)
            nc.sync.dma_start(out=w1x, in_=w1x_d)
            w1e = wpool.tile([24, 128], BF16)
            nc.sync.dma_start(out=w1e, in_=w1e_d)
            w2 = wpool.tile([128, 3, 64], BF16)
            nc.sync.dma_start(out=w2, in_=w2_d)
            w3 = wpool.tile([128, 3, 4], BF16)
            nc.sync.dma_start(out=w3, in_=w3_d)
            gmask = wpool.tile([128, 8], F32)
            nc.sync.dma_start(out=gmask, in_=gm_d)
            expand = wpool.tile([8, 128], F32)
            nc.sync.dma_start(out=expand, in_=ex_d)
            vecs = wpool.tile([128, 4], F32)
            nc.sync.dma_start(out=vecs, in_=vec_d)
            halfv = wpool.tile([1, 128], F32)
            nc.sync.dma_start(out=halfv, in_=half_d)
            ones2 = wpool.tile([2, 128], F32)
            nc.sync.dma_start(out=ones2, in_=ones2_d)
            gate = wpool.tile([1, 1], F32)
            nc.sync.dma_start(out=gate, in_=gate_d)
            magic = wpool.tile([8, 1], mybir.dt.uint32)
            nc.vector.memset(magic, 0x5f3759df)

            # ---- residual constant: cvec[b] = 0.5*tanh(gate)*(x[b,2047]+x[b,2048])
            taus = wpool.tile([1, 1], F32)
            nc.scalar.activation(taus, gate, AF.Tanh)
            ps_t = pstp.tile([128, 1], F32, tag="st")
            nc.tensor.matmul(ps_t, halfv, taus, start=True, stop=True)
            tau128 = wpool.tile([128, 1], F32)
            nc.vector.tensor_copy(tau128, ps_t)          # 0.5*tanh(gate) on all parts

            cvecs = []
            for b in range(BPC):
                xrows = wpool.tile([2, C], F32, tag=f"xrows{b}")
                nc.sync.dma_start(out=xrows, in_=x_d[b, 2047:2049, :])
                ps_cv = pstp.tile([128, C], F32, tag="st")
                nc.tensor.matmul(ps_cv, ones2, xrows, start=True, stop=True)
                cv = wpool.tile([128, 4, LT], F32, tag=f"cvec{b}")
                nc.vector.tensor_scalar(cv[:, 0, :], ps_cv, tau128, None, op0=ALU.mult)
                for kk in range(1, 4):
                    nc.vector.tensor_copy(cv[:, kk, :], cv[:, 0, :])
                cvecs.append(cv)

            # residual views: [8, 128, 4, 512] macro-tiles per batch
            xr = [x_d[b].rearrange("(mt ch p) c -> mt p ch c", ch=4, p=128)
                  for b in range(BPC)]
            o0r = [out0_d[b].rearrange("(mt ch p) c -> mt p ch c", ch=4, p=128)
                   for b in range(BPC)]

            def emit_residual(mt_global):
                # rt <- cvec broadcast; then DMA x with inline accumulate (CCE
                # add in the SDMA datapath); store. No compute-engine pass.
                b, mt = mt_global // 8, mt_global % 8
                rt = resp.tile([128, 4, LT], F32, tag="res")
                nc.vector.tensor_copy(rt, cvecs[b])
                nc.gpsimd.dma_start(out=rt, in_=xr[b][mt], accum_op=ALU.add)
                nc.scalar.dma_start(out=o0r[b][mt], in_=rt)

            def conv1_pair(p):
                """conv1 + bn_stats for pair p -> (h1 tile, stats tile)."""
                b_idx = p // 4
                ch0 = 128 * (p % 4)
                j0 = 2 * p
                xt = xtp.tile([128, L + 2], BF16, tag="xt")
                nc.gpsimd.memset(xt[:, 0:1], 0.0)
                nc.gpsimd.memset(xt[:, L + 1:L + 2], 0.0)
                nc.sync.dma_start(out=xt[:, 1:L + 1], in_=xT_d[b_idx, ch0:ch0 + 128, :])
                xe = xep.tile([24, L], BF16, tag="xe")
                nc.sync.dma_start(out=xe, in_=xe_d[j0:j0 + 2].rearrange("a b l -> (a b) l"))
                h1t = h1p.tile([128, L + 2], BF16, tag="h1")
                nc.gpsimd.memset(h1t[:, 0:1], 0.0)
                nc.gpsimd.memset(h1t[:, L + 1:L + 2], 0.0)
                stats = stp.tile([128, NLT, 6], F32, tag="stats", bufs=4)
                for lt in range(NLT):
                    ps = ps1p.tile([128, LT], F32, tag="ps1")
                    for k in range(3):
                        nc.tensor.matmul(ps, w1x[:, k, :],
                                         xt[:, lt * LT + k: lt * LT + k + LT],
                                         start=(k == 0), stop=False)
                    nc.tensor.matmul(ps, w1e, xe[:, lt * LT:(lt + 1) * LT],
                                     start=False, stop=True)
                    dst = h1t[:, 1 + lt * LT: 1 + (lt + 1) * LT]
                    if lt % 2 == 0:
                        nc.vector.tensor_copy(dst, ps)
                    else:
                        nc.scalar.activation(dst, ps, AF.Copy)
                    nc.vector.bn_stats(stats[:, lt, :], dst)
                return h1t, stats

            def stats_gelu(h1t, stats):
                """GroupNorm affine (b1 folded) + gelu, in place on h1t."""
                mv = stp.tile([128, 2], F32, tag="mv")
                nc.vector.bn_aggr(mv, stats)
                svec = stp.tile([128, 2], F32, tag="svec")
                nc.vector.tensor_add(svec[:, 0:1], mv[:, 0:1], vecs[:, 0:1])
                sq = stp.tile([128, 1], F32, tag="sq")
                nc.vector.tensor_mul(sq, svec[:, 0:1], svec[:, 0:1])
                nc.vector.tensor_add(svec[:, 1:2], mv[:, 1:2], sq)
                psg = pstp.tile([8, 2], F32, tag="st")
                nc.tensor.matmul(psg, gmask, svec, start=True, stop=True)
                gv = stp.tile([8, 2], F32, tag="gv")
                nc.vector.tensor_copy(gv, psg)
                musq = stp.tile([8, 1], F32, tag="musq")
                nc.vector.tensor_mul(musq, gv[:, 0:1], gv[:, 0:1])
                varg = stp.tile([8, 1], F32, tag="varg")
                nc.vector.tensor_sub(varg, gv[:, 1:2], musq)
                # rstd = 1/sqrt(varg + eps), table-free (Quake seed + Newton)
                nc.vector.tensor_scalar(varg, varg, EPS, None, op0=ALU.add)
                vh = stp.tile([8, 1], F32, tag="vh")
                nc.vector.tensor_scalar(vh, varg, 0.5, None, op0=ALU.mult)
                yb = stp.tile([8, 1], mybir.dt.uint32, tag="yb")
                nc.vector.tensor_scalar(yb, varg.bitcast(mybir.dt.uint32), 1, None,
                                        op0=ALU.logical_shift_right)
                nc.vector.tensor_sub(yb, magic, yb)
                rhs2 = stp.tile([8, 2], F32, tag="rhs2")
                y = yb.bitcast(F32)
                tq = stp.tile([8, 1], F32, tag="tq")
                for it in range(3):
                    dst = rhs2[:, 0:1] if it == 2 else y
                    nc.vector.tensor_mul(tq, y, y)
                    nc.vector.tensor_mul(tq, tq, vh)
                    nc.vector.tensor_scalar(tq, tq, 1.5, -1.0,
                                            op0=ALU.subtract, op1=ALU.mult)
                    nc.vector.tensor_mul(dst, y, tq)
                nc.vector.tensor_copy(rhs2[:, 1:2], gv[:, 0:1])
                psp = pstp.tile([128, 2], F32, tag="st")
                nc.tensor.matmul(psp, expand, rhs2, start=True, stop=True)
                ap_ = stp.tile([128, 1], F32, tag="ap_")
                bp_ = stp.tile([128, 1], F32, tag="bp_")
                tmp = stp.tile([128, 1], F32, tag="tmp")
                nc.vector.tensor_mul(ap_, psp[:, 0:1], vecs[:, 1:2])
                nc.vector.tensor_sub(tmp, vecs[:, 0:1], psp[:, 1:2])
                nc.vector.tensor_mul(tmp, tmp, ap_)
                nc.vector.tensor_add(bp_, tmp, vecs[:, 2:3])
                nc.scalar.activation(h1t[:, 1:L + 1], h1t[:, 1:L + 1], AF.Gelu,
                                     bias=bp_, scale=ap_)

            def conv3_lt(h2q, q, lt):
                ps3 = ps3p.tile([4, LT], F32, tag="ps3")
                for k in range(3):
                    nc.tensor.matmul(ps3, w3[:, k, :],
                                     h2q[:, lt * LT + k: lt * LT + k + LT],
                                     start=(k == 0), stop=(k == 2))
                ot = offp.tile([4, LT], F32, tag="off")
                nc.scalar.activation(ot, ps3, AF.Tanh)
                nc.vector.tensor_scalar(ot, ot, 10.0, None, op0=ALU.mult)
                nc.sync.dma_start(out=out1_d[4 * q:4 * q + 4, lt * LT:(lt + 1) * LT],
                                  in_=ot)

            def conv_part(q):
                """residual tiles + conv1 (+ per-tile stats) for quad q."""
                for mt in range(4 * q, 4 * q + 4):
                    emit_residual(mt)
                h1a, sta = conv1_pair(2 * q)
                h1b, stb = conv1_pair(2 * q + 1)
                return (h1a, sta), (h1b, stb)

            def stats_part(pairs):
                for h1t, st in pairs:
                    stats_gelu(h1t, st)

            def phase2(q, h1a, h1b):
                h2q = h2p.tile([128, L + 2], BF16, tag="h2")
                nc.gpsimd.memset(h2q[:, 0:1], 0.0)
                nc.gpsimd.memset(h2q[:, L + 1:L + 2], 0.0)
                for lt in range(NLT):
                    ps2 = ps2p.tile([128, LT], F32, tag="ps2")
                    for half, h1t in enumerate((h1a, h1b)):
                        tp = (0, 64 * half) if half else None
                        for k in range(3):
                            nc.tensor.matmul(ps2[64 * half:64 * half + 64, :],
                                             w2[:, k, :],
                                             h1t[:, lt * LT + k: lt * LT + k + LT],
                                             start=(k == 0), stop=(k == 2),
                                             tile_position=tp)
                    nc.scalar.activation(h2q[:, 1 + lt * LT: 1 + (lt + 1) * LT],
                                         ps2, AF.Gelu, bias=vecs[:, 3:4], scale=1.0)
                    if lt >= 1:
                        conv3_lt(h2q, q, lt - 1)
                conv3_lt(h2q, q, NLT - 1)

            # software pipeline: PE order is conv1(q+1) -> stat-MMs(q) ->
            # conv2/conv3(q), so the DVE stat chains for quad q run while PE
            # does conv1(q+1) and the PE never waits on them.
            pend = {0: conv_part(0)}
            for q in range(4):
                if q + 1 < 4:
                    pend[q + 1] = conv_part(q + 1)
                pairs = pend.pop(q)
                stats_part(pairs)
                phase2(q, pairs[0][0], pairs[1][0])

    nc.finalize()
    return nc


_CACHE = {}


def _get_program():
    if "nc" not in _CACHE:
        _CACHE["nc"] = _build_program()
    return _CACHE["nc"]


def kernel(x, x_ext, w1, b1, gn_g, gn_b, w2, b2, w3, gate, trace=False):
    x = np.asarray(x, np.float32)
    in_maps = _host_prep(x, np.asarray(x_ext, np.float32),
                         np.asarray(w1, np.float32), np.asarray(b1, np.float32),
                         np.asarray(gn_g, np.float32), np.asarray(gn_b, np.float32),
                         np.asarray(w2, np.float32), np.asarray(b2, np.float32),
                         np.asarray(w3, np.float32), np.asarray(gate, np.float32))
    nc = _get_program()
    res = run_bass_kernel_spmd(nc, in_maps, core_ids=list(range(NCORES)),
                               trace=trace)
    out0 = np.concatenate([res.results[c]["out0"] for c in range(NCORES)], axis=0)
    out1 = np.concatenate([res.results[c]["out1"] for c in range(NCORES)], axis=0)
    out1 = out1.reshape(B * G, 1, L)
    if trace:
        kernel.last_result = res
    return out0, out1


# revision 38
# speedup vs baseline: 1.6763x; 1.0004x over previous
"""Trainium2 Bass kernel for nn_ClimateAwareDeformableAligner.

Reference semantics (verified numerically):
  - The grid_sample receives input [N, C, H=L, W=1]; the computed grid x-coord
    indexes the size-1 W axis (multiplied by W-1 = 0) and the y-coord is 0 ->
    iy = (L-1)/2 = 2047.5. The whole deformable gather collapses to
        xs[n, c, l] = 0.5 * (xg[n, c, 2047] + xg[n, c, 2048])   (const along l)
    so out0 = x + tanh(gate) * 0.5 * (x[:, 2047, :] + x[:, 2048, :]).
  - offset (second output) needs the full conv stack:
        conv1 -> GroupNorm(4) -> gelu -> conv2 -> gelu -> conv3 -> 10*tanh.

Sharding: data-parallel over batch. 8 cores x 2 batches each; the fused
B*G = 128 rows split as 16 consecutive rows per core.
"""
import ml_dtypes
import numpy as np

import concourse.bacc as bacc
import concourse.bass as bass
import concourse.tile as tile
from concourse import mybir
from concourse.bass_utils import run_bass_kernel_spmd

F32 = mybir.dt.float32
BF16 = mybir.dt.bfloat16
NPBF16 = ml_dtypes.bfloat16
AF = mybir.ActivationFunctionType
ALU = mybir.AluOpType

B, L, C, G, CG, WD = 16, 4096, 512, 8, 64, 4
NCORES = 8
BPC = B // NCORES          # batches per core = 2
NPC = BPC * G              # n rows per core = 16
LT = 512                   # L tile (psum bank)
NLT = L // LT              # 8
EPS = 1e-5


# ---------------------------------------------------------------- host prep
def _host_prep(x, x_ext, w1, b1, gn_g, gn_b, w2, b2, w3, gate):
    xT = np.ascontiguousarray(x.transpose(0, 2, 1))           # [16, 512, 4096]

    xe_pad = np.zeros((B, L + 2, WD), np.float32)
    xe_pad[:, 1:L + 1, :] = x_ext
    xe_col = np.empty((B, 12, L), np.float32)
    for k in range(3):
        for wd in range(WD):
            xe_col[:, k * 4 + wd, :] = xe_pad[:, k:k + L, wd]

    # stacked (not diag) conv1 weights: both 64-row halves hold the same
    # [ci, co] block so the four quad matmuls can use either partition half.
    w1x = np.zeros((128, 3, 64), np.float32)                  # [ci, k, co]
    for k in range(3):
        blk = w1[:, :64, k].T
        w1x[:64, k, :] = blk
        w1x[64:, k, :] = blk

    w1e_flat = np.zeros((12, 64), np.float32)
    for k in range(3):
        for wd in range(WD):
            w1e_flat[k * 4 + wd, :] = w1[:, 64 + wd, k]
    w1e = np.zeros((24, 128), np.float32)          # pair A: [n0|n1] psum order
    w1e[:12, :64] = w1e_flat
    w1e[12:, 64:] = w1e_flat
    w1eB = np.zeros((24, 128), np.float32)         # pair B: swapped [n1|n0]
    w1eB[:12, 64:] = w1e_flat
    w1eB[12:, :64] = w1e_flat

    w2b = np.zeros((128, 3, 64), np.float32)
    for k in range(3):
        blk = w2[:, :, k].T
        w2b[:64, k, :32] = blk
        w2b[64:, k, 32:] = blk

    # h2 quad partition blocks are [A-n0, A-n1, B-n1, B-n0] (pair B swapped);
    # permute conv3 output columns so out1 rows are n_local order.
    w3b = np.zeros((128, 3, 4), np.float32)
    for k in range(3):
        for m, col in ((0, 0), (1, 1), (2, 3), (3, 2)):
            w3b[32 * m:32 * m + 32, k, col] = w3[0, :, k]

    gmask = np.zeros((128, 8), np.float32)
    expand = np.zeros((8, 128), np.float32)
    for p in range(128):
        gmask[p, p // 16] = 1.0 / 16.0
        expand[p // 16, p] = 1.0

    vecs = np.stack([np.tile(b1, 2), np.tile(gn_g, 2),
                     np.tile(gn_b, 2), np.tile(b2, 4)], axis=1).astype(np.float32)

    halfv = np.full((1, 128), 0.5, np.float32)
    ones2 = np.ones((2, 128), np.float32)
    gate2 = np.asarray(gate, np.float32).reshape(1, 1)

    common = dict(xe=xe_col.astype(NPBF16), w1x=w1x.astype(NPBF16),
                  w1e=w1e.astype(NPBF16), w1eB=w1eB.astype(NPBF16),
                  w2=w2b.astype(NPBF16), w3=w3b.astype(NPBF16),
                  gmask=gmask, expand=expand, vecs=vecs,
                  halfv=halfv, ones2=ones2, gate=gate2)
    xTb = xT.astype(NPBF16)
    in_maps = []
    for c in range(NCORES):
        m = dict(common)
        m["x"] = np.ascontiguousarray(x[2 * c:2 * c + 2])
        m["xT"] = np.ascontiguousarray(xTb[2 * c:2 * c + 2])
        in_maps.append(m)
    return in_maps


# ---------------------------------------------------------------- program
def _build_program():
    nc = bacc.Bacc("TRN2", target_bir_lowering=False, debug=False)

    x_d = nc.dram_tensor("x", [BPC, L, C], F32, kind="ExternalInput")[:]
    xT_d = nc.dram_tensor("xT", [BPC, C, L], BF16, kind="ExternalInput")[:]
    xe_d = nc.dram_tensor("xe", [B, 12, L], BF16, kind="ExternalInput")[:]
    w1x_d = nc.dram_tensor("w1x", [128, 3, 64], BF16, kind="ExternalInput")[:]
    w1e_d = nc.dram_tensor("w1e", [24, 128], BF16, kind="ExternalInput")[:]
    w1eB_d = nc.dram_tensor("w1eB", [24, 128], BF16, kind="ExternalInput")[:]
    w2_d = nc.dram_tensor("w2", [128, 3, 64], BF16, kind="ExternalInput")[:]
    w3_d = nc.dram_tensor("w3", [128, 3, 4], BF16, kind="ExternalInput")[:]
    gm_d = nc.dram_tensor("gmask", [128, 8], F32, kind="ExternalInput")[:]
    ex_d = nc.dram_tensor("expand", [8, 128], F32, kind="ExternalInput")[:]
    vec_d = nc.dram_tensor("vecs", [128, 4], F32, kind="ExternalInput")[:]
    half_d = nc.dram_tensor("halfv", [1, 128], F32, kind="ExternalInput")[:]
    ones2_d = nc.dram_tensor("ones2", [2, 128], F32, kind="ExternalInput")[:]
    gate_d = nc.dram_tensor("gate", [1, 1], F32, kind="ExternalInput")[:]

    out0_d = nc.dram_tensor("out0", [BPC, L, C], F32, kind="ExternalOutput")[:]
    out1_d = nc.dram_tensor("out1", [NPC, L], F32, kind="ExternalOutput")[:]

    with tile.TileContext(nc) as tc:
        with tc.tile_pool(name="wpool", bufs=1) as wpool, \
             tc.tile_pool(name="xtp", bufs=2) as xtp, \
             tc.tile_pool(name="xep", bufs=2) as xep, \
             tc.tile_pool(name="h1p", bufs=4) as h1p, \
             tc.tile_pool(name="h2p", bufs=2) as h2p, \
             tc.tile_pool(name="resp", bufs=4) as resp, \
             tc.tile_pool(name="offp", bufs=2) as offp, \
             tc.tile_pool(name="stp", bufs=2) as stp, \
             tc.tile_pool(name="ps1p", bufs=3, space="PSUM") as ps1p, \
             tc.tile_pool(name="ps2p", bufs=2, space="PSUM") as ps2p, \
             tc.tile_pool(name="ps3p", bufs=2, space="PSUM") as ps3p, \
             tc.tile_pool(name="pstp", bufs=1, space="PSUM") as pstp:

            # ---- constants
            w1x = wpool.tile([128, 3, 64], BF16)
            nc.sync.dma_start(out=w1x, in_=w1x_d)
            w1e = wpool.tile([24, 128], BF16)
            nc.sync.dma_start(out=w1e, in_=w1e_d)
            w1eB = wpool.tile([24, 128], BF16)
            nc.sync.dma_start(out=w1eB, in_=w1eB_d)
            w2 = wpool.tile([128, 3, 64], BF16)
            nc.sync.dma_start(out=w2, in_=w2_d)
            w3 = wpool.tile([128, 3, 4], BF16)
            nc.sync.dma_start(out=w3, in_=w3_d)
            gmask = wpool.tile([128, 8], F32)
            nc.sync.dma_start(out=gmask, in_=gm_d)
            expand = wpool.tile([8, 128], F32)
            nc.sync.dma_start(out=expand, in_=ex_d)
            vecs = wpool.tile([128, 4], F32)
            nc.sync.dma_start(out=vecs, in_=vec_d)
            halfv = wpool.tile([1, 128], F32)
            nc.sync.dma_start(out=halfv, in_=half_d)
            ones2 = wpool.tile([2, 128], F32)
            nc.sync.dma_start(out=ones2, in_=ones2_d)
            gate = wpool.tile([1, 1], F32)
            nc.sync.dma_start(out=gate, in_=gate_d)
            magic = wpool.tile([8, 1], mybir.dt.uint32)
            nc.vector.memset(magic, 0x5f3759df)

            # ---- residual constant: cvec[b] = 0.5*tanh(gate)*(x[b,2047]+x[b,2048])
            cvecs = []

            def cvec_setup():
                taus = wpool.tile([1, 1], F32)
                nc.scalar.activation(taus, gate, AF.Tanh)
                ps_t = pstp.tile([128, 1], F32, tag="st")
                nc.tensor.matmul(ps_t, halfv, taus, start=True, stop=True)
                tau128 = wpool.tile([128, 1], F32)
                nc.vector.tensor_copy(tau128, ps_t)      # 0.5*tanh(gate), all parts
                for b in range(BPC):
                    xrows = wpool.tile([2, C], F32, tag=f"xrows{b}")
                    nc.sync.dma_start(out=xrows, in_=x_d[b, 2047:2049, :])
                    ps_cv = pstp.tile([128, C], F32, tag="st")
                    nc.tensor.matmul(ps_cv, ones2, xrows, start=True, stop=True)
                    cv = wpool.tile([128, 4, LT], F32, tag=f"cvec{b}")
                    nc.vector.tensor_scalar(cv[:, 0, :], ps_cv, tau128, None,
                                            op0=ALU.mult)
                    for kk in range(1, 4):
                        nc.vector.tensor_copy(cv[:, kk, :], cv[:, 0, :])
                    cvecs.append(cv)

            # residual views: [8, 128, 4, 512] macro-tiles per batch
            xr = [x_d[b].rearrange("(mt ch p) c -> mt p ch c", ch=4, p=128)
                  for b in range(BPC)]
            o0r = [out0_d[b].rearrange("(mt ch p) c -> mt p ch c", ch=4, p=128)
                   for b in range(BPC)]

            def emit_residual(mt_global):
                b, mt = mt_global // 8, mt_global % 8
                rt = resp.tile([128, 4, LT], F32, tag="res")
                nc.scalar.dma_start(out=rt, in_=xr[b][mt])
                nc.gpsimd.tensor_tensor(out=rt, in0=rt, in1=cvecs[b], op=ALU.add)
                nc.scalar.dma_start(out=o0r[b][mt], in_=rt)

            def _load_pair(p):
                b_idx = p // 4
                ch0 = 128 * (p % 4)
                j0 = 2 * p
                xt = xtp.tile([128, L + 2], BF16, tag="xt")
                nc.gpsimd.memset(xt[:, 0:1], 0.0)
                nc.gpsimd.memset(xt[:, L + 1:L + 2], 0.0)
                nc.sync.dma_start(out=xt[:, 1:L + 1], in_=xT_d[b_idx, ch0:ch0 + 128, :])
                xe = xep.tile([24, L], BF16, tag="xe")
                nc.sync.dma_start(out=xe, in_=xe_d[j0:j0 + 2].rearrange("a b l -> (a b) l"))
                h1t = h1p.tile([128, L + 2], BF16, tag="h1")
                nc.gpsimd.memset(h1t[:, 0:1], 0.0)
                nc.gpsimd.memset(h1t[:, L + 1:L + 2], 0.0)
                stats = stp.tile([128, NLT, 6], F32, tag="stats", bufs=4)
                return xt, xe, h1t, stats

            def conv1_quad(q):
                """conv1 + bn_stats for both pairs of quad q, 4-way packed on
                the PE's 32x32 subarray grid. Pair B's psum layout is swapped
                ([n1|n0]); downstream GN/conv2 are layout-invariant and conv3's
                w3 has permuted output columns to compensate."""
                xtA, xeA, h1a, sta = _load_pair(2 * q)
                xtB, xeB, h1b, stb = _load_pair(2 * q + 1)
                for lt in range(NLT):
                    psA = ps1p.tile([128, LT], F32, tag="ps1")
                    psB = ps1p.tile([128, LT], F32, tag="ps1")
                    for k in range(3):
                        sl = slice(lt * LT + k, lt * LT + k + LT)
                        st = (k == 0)
                        nc.tensor.matmul(psA[0:64, :], w1x[0:64, k, :],
                                         xtA[0:64, sl], start=st, stop=False,
                                         tile_position=(0, 0),
                                         skip_group_check=True)
                        nc.tensor.matmul(psA[64:128, :], w1x[64:128, k, :],
                                         xtA[64:128, sl], start=st, stop=False,
                                         tile_position=(64, 64),
                                         skip_group_check=True)
                        nc.tensor.matmul(psB[0:64, :], w1x[64:128, k, :],
                                         xtB[64:128, sl], start=st, stop=False,
                                         tile_position=(64, 0),
                                         skip_group_check=True)
                        nc.tensor.matmul(psB[64:128, :], w1x[0:64, k, :],
                                         xtB[0:64, sl], start=st, stop=False,
                                         tile_position=(0, 64),
                                         skip_group_check=True)
                    lsl = slice(lt * LT, (lt + 1) * LT)
                    nc.tensor.matmul(psA, w1e, xeA[:, lsl], start=False,
                                     stop=True, skip_group_check=True)
                    nc.tensor.matmul(psB, w1eB, xeB[:, lsl], start=False,
                                     stop=True, skip_group_check=True)
                    for ps, h1t, stt in ((psA, h1a, sta), (psB, h1b, stb)):
                        dst = h1t[:, 1 + lt * LT: 1 + (lt + 1) * LT]
                        nc.vector.tensor_copy(dst, ps)
                        nc.vector.bn_stats(stt[:, lt, :], dst)
                return (h1a, sta), (h1b, stb)

            def stats_gelu(h1t, stats):
                """GroupNorm affine (b1 folded) + gelu, in place on h1t."""
                mv = stp.tile([128, 2], F32, tag="mv")
                nc.vector.bn_aggr(mv, stats)
                svec = stp.tile([128, 2], F32, tag="svec")
                nc.vector.tensor_add(svec[:, 0:1], mv[:, 0:1], vecs[:, 0:1])
                sq = stp.tile([128, 1], F32, tag="sq")
                nc.vector.tensor_mul(sq, svec[:, 0:1], svec[:, 0:1])
                nc.vector.tensor_add(svec[:, 1:2], mv[:, 1:2], sq)
                psg = pstp.tile([8, 2], F32, tag="st")
                nc.tensor.matmul(psg, gmask, svec, start=True, stop=True)
                gv = stp.tile([8, 2], F32, tag="gv")
                nc.vector.tensor_copy(gv, psg)
                musq = stp.tile([8, 1], F32, tag="musq")
                nc.vector.tensor_mul(musq, gv[:, 0:1], gv[:, 0:1])
                varg = stp.tile([8, 1], F32, tag="varg")
                nc.vector.tensor_sub(varg, gv[:, 1:2], musq)
                # rstd = 1/sqrt(varg + eps), table-free (Quake seed + Newton)
                nc.vector.tensor_scalar(varg, varg, EPS, None, op0=ALU.add)
                vh = stp.tile([8, 1], F32, tag="vh")
                nc.vector.tensor_scalar(vh, varg, 0.5, None, op0=ALU.mult)
                yb = stp.tile([8, 1], mybir.dt.uint32, tag="yb")
                nc.vector.tensor_scalar(yb, varg.bitcast(mybir.dt.uint32), 1, None,
                                        op0=ALU.logical_shift_right)
                nc.vector.tensor_sub(yb, magic, yb)
                rhs2 = stp.tile([8, 2], F32, tag="rhs2")
                y = yb.bitcast(F32)
                tq = stp.tile([8, 1], F32, tag="tq")
                for it in range(3):
                    dst = rhs2[:, 0:1] if it == 2 else y
                    nc.vector.tensor_mul(tq, y, y)
                    nc.vector.tensor_mul(tq, tq, vh)
                    nc.vector.tensor_scalar(tq, tq, 1.5, -1.0,
                                            op0=ALU.subtract, op1=ALU.mult)
                    nc.vector.tensor_mul(dst, y, tq)
                nc.vector.tensor_copy(rhs2[:, 1:2], gv[:, 0:1])
                psp = pstp.tile([128, 2], F32, tag="st")
                nc.tensor.matmul(psp, expand, rhs2, start=True, stop=True)
                ap_ = stp.tile([128, 1], F32, tag="ap_")
                bp_ = stp.tile([128, 1], F32, tag="bp_")
                tmp = stp.tile([128, 1], F32, tag="tmp")
                nc.vector.tensor_mul(ap_, psp[:, 0:1], vecs[:, 1:2])
                nc.vector.tensor_sub(tmp, vecs[:, 0:1], psp[:, 1:2])
                nc.vector.tensor_mul(tmp, tmp, ap_)
                nc.vector.tensor_add(bp_, tmp, vecs[:, 2:3])
                # split gelu so conv2 of the first half can start earlier
                half_l = L // 2
                for hh in range(2):
                    sl = slice(1 + hh * half_l, 1 + (hh + 1) * half_l)
                    nc.scalar.activation(h1t[:, sl], h1t[:, sl], AF.Gelu,
                                         bias=bp_, scale=ap_)

            def conv3_lt(h2q, q, lt):
                ps3 = ps3p.tile([4, LT], F32, tag="ps3")
                for k in range(3):
                    nc.tensor.matmul(ps3, w3[:, k, :],
                                     h2q[:, lt * LT + k: lt * LT + k + LT],
                                     start=(k == 0), stop=(k == 2))
                ot = offp.tile([4, LT], F32, tag="off")
                nc.scalar.activation(ot, ps3, AF.Tanh)
                nc.vector.tensor_scalar(ot, ot, 10.0, None, op0=ALU.mult)
                nc.sync.dma_start(out=out1_d[4 * q:4 * q + 4, lt * LT:(lt + 1) * LT],
                                  in_=ot)

            def stats_part(pairs):
                for h1t, st in pairs:
                    stats_gelu(h1t, st)

            def phase2(q, h1a, h1b):
                for mt in range(4 * q, 4 * q + 4):
                    emit_residual(mt)
                h2q = h2p.tile([128, L + 2], BF16, tag="h2")
                nc.gpsimd.memset(h2q[:, 0:1], 0.0)
                nc.gpsimd.memset(h2q[:, L + 1:L + 2], 0.0)
                for lt in range(NLT):
                    ps2 = ps2p.tile([128, LT], F32, tag="ps2")
                    for half, h1t in enumerate((h1a, h1b)):
                        tp = (0, 64 * half) if half else None
                        for k in range(3):
                            nc.tensor.matmul(ps2[64 * half:64 * half + 64, :],
                                             w2[:, k, :],
                                             h1t[:, lt * LT + k: lt * LT + k + LT],
                                             start=(k == 0), stop=(k == 2),
                                             tile_position=tp)
                    nc.scalar.activation(h2q[:, 1 + lt * LT: 1 + (lt + 1) * LT],
                                         ps2, AF.Gelu, bias=vecs[:, 3:4], scale=1.0)
                    if lt >= 1:
                        conv3_lt(h2q, q, lt - 1)
                conv3_lt(h2q, q, NLT - 1)

            # software pipeline: PE order is conv1(q+1) -> stat-MMs(q) ->
            # conv2/conv3(q), so the DVE stat chains for quad q run while PE
            # does conv1(q+1) and the PE never waits on them. The cvec setup
            # is emitted after conv1(0) so the first PE op isn't gated on it.
            pend = {0: conv1_quad(0)}
            cvec_setup()
            for q in range(4):
                if q + 1 < 4:
                    pend[q + 1] = conv1_quad(q + 1)
                pairs = pend.pop(q)
                stats_part(pairs)
                phase2(q, pairs[0][0], pairs[1][0])

    nc.finalize()
    return nc


_CACHE = {}


def _get_program():
    if "nc" not in _CACHE:
        _CACHE["nc"] = _build_program()
    return _CACHE["nc"]


def kernel(x, x_ext, w1, b1, gn_g, gn_b, w2, b2, w3, gate, trace=False):
    x = np.asarray(x, np.float32)
    in_maps = _host_prep(x, np.asarray(x_ext, np.float32),
                         np.asarray(w1, np.float32), np.asarray(b1, np.float32),
                         np.asarray(gn_g, np.float32), np.asarray(gn_b, np.float32),
                         np.asarray(w2, np.float32), np.asarray(b2, np.float32),
                         np.asarray(w3, np.float32), np.asarray(gate, np.float32))
    nc = _get_program()
    res = run_bass_kernel_spmd(nc, in_maps, core_ids=list(range(NCORES)),
                               trace=trace)
    out0 = np.concatenate([res.results[c]["out0"] for c in range(NCORES)], axis=0)
    out1 = np.concatenate([res.results[c]["out1"] for c in range(NCORES)], axis=0)
    out1 = out1.reshape(B * G, 1, L)
    if trace:
        kernel.last_result = res
    return out0, out1
